# revision 11
# baseline (speedup 1.0000x reference)
"""Trainium2 Bass kernel for CIN (Compressed Interaction Network) forward.

Reference computation (per batch b, per dim d, with x = inputs[b, :, d], F=32):
  z0[(h,m)] = x[h]*x[m]                    (1024-vector)
  y0 = relu(W0 @ z0 + b0)                  (128)
  h1 = y0[:64]; f0 = y0[64:]
  z1[(g,m)] = h1[g]*x[m]                   (2048-vector)
  f1 = relu(W1 @ z1 + b1)                  (128)
  out[b, 0:64]  = sum_d f0
  out[b, 64:192] = sum_d f1

Strategy: pure data parallel over 8 cores (256 batch each). Per core the
(b, d) pairs form 16384 GEMM columns, processed in 16 chunks of 1024.

The outer-product feature tiles z are built with PLAIN tensor_tensor
multiplies (which run in the DVE's 2x packed-fp16 mode, 2 elem/lane/cycle)
against host-precomputed partition-rotated / partition-broadcast operand
tiles streamed from HBM:
  z0 chunk c: xr0[:, c] * xa      (xr0 = 5 host-rotated variants of x)
  z1 chunk c: xb[:, c] * y2d      (xb = 16 host-broadcast m-row variants;
                                   multiplied IN PLACE over the xb tile)
A slice of the 16 z1 chunks is built on the otherwise-idle GpSimd (Pool)
engine to offload the Vector engine. GEMMs run on the Tensor engine in
fp16, layer-0 exploiting z0 symmetry (5 chunks of 128 contraction rows
instead of 8). relu+bias runs on the Scalar engine; the per-batch
d-reduction runs as one windowed tensor_reduce per layer per chunk on the
Vector engine (replacing 32 tiny accum activations). The final
(channel, batch) -> (batch, channel) transpose runs on the Tensor engine.
The chunk pipeline is software-pipelined two deep so every engine stays
busy; input tiles prefetch one chunk ahead on the Sync + Scalar DMA queues.
"""

import sys

sys.path.insert(0, "/opt/trn_rl_repo")

import numpy as np

import concourse.bass as bass
import concourse.mybir as mybir
import concourse.tile as tile
from concourse import bacc
from concourse.bass_utils import run_bass_kernel_spmd
from concourse.masks import make_identity

# ---- problem constants (hardcoded per contract) ---------------------------- #
B = 2048
F = 32  # field size (channels in)
D = 64  # embedding dim
O0 = 128  # layer-0 out channels
O1 = 128  # layer-1 out channels
H1 = 64  # split half fed to layer 1
NCORES = 8
BC = B // NCORES  # batch per core
NCHUNK = 1024  # GEMM columns per chunk (16 batch x 64 d)
BPC = NCHUNK // D  # batch elems per chunk
NCHUNKS = BC * D // NCHUNK
L0C = 5  # layer-0 z chunks (symmetric cover: difference classes 0..16)
L0_SHIFT = (0, 4, 8, 12, 16)  # per-chunk lane shift for the z0 row map
L1C = 16  # layer-1 z chunks (2048 rows / 128)
MMF = 512  # matmul free-dim per instruction
DT = mybir.dt.float16
FP32 = mybir.dt.float32

# ---- schedule knobs -------------------------------------------------------- #
# layer-1 chunk assignment: chunks in DVE_GROUPS run on the Vector engine
# (grouped ops, 2x mode); POOL_CHUNKS run on GpSimd.
DVE_GROUPS = ((0, 4), (4, 8), (8, 11))
POOL_CHUNKS = tuple(range(11, 16))
# xb prefetch DMA issue engine per chunk: first half via scalar queue,
# second half via sync queue (xa+xr0 also ride the sync queue).
XB_SPLIT = 8  # chunks < XB_SPLIT issue from scalar queue, rest from sync

# ---- host-side row maps ---------------------------------------------------- #
# z0 chunk c, row 32q+l  holds pair {h, m} = {(l + L0_SHIFT[c] + q) % 32, l}
# z1 chunk c, row 32q+l  holds pair (g, m) = (32*(q%2) + l, (c + 16*(q//2)) % 32)

_Q = np.arange(128) // 32
_L = np.arange(128) % 32
ROWS_XA = _L.copy()  # xa row map (x replicated per quadrant)
ROWS_XR0 = np.empty((128, L0C), dtype=np.int64)
for _c in range(L0C):
    ROWS_XR0[:, _c] = (_L + L0_SHIFT[_c] + _Q) % 32
ROWS_XB = np.empty((128, L1C), dtype=np.int64)
for _c in range(L1C):
    ROWS_XB[:, _c] = (_c + 16 * (_Q // 2)) % 32


def _prep_weights(W0, b0, W1, b1):
    w0 = W0.reshape(O0, F, F)  # [o, h, m]
    w0sym = w0 + w0.transpose(0, 2, 1)
    w0t = np.zeros((L0C, 128, O0), dtype=np.float16)
    for c in range(L0C):
        for q in range(4):
            delta = L0_SHIFT[c] + q
            if delta > 16:
                continue  # duplicate class, keep zero weights
            for l in range(32):
                if delta == 16 and l >= 16:
                    continue  # delta=16 pairs appear twice; keep first half
                h = (l + delta) % 32
                if delta == 0:
                    w0t[c, 32 * q + l, :] = w0[:, l, l].astype(np.float16)
                else:
                    w0t[c, 32 * q + l, :] = w0sym[:, h, l].astype(np.float16)
    w1 = W1.reshape(O1, H1, F)  # [o, g, m]
    w1t = np.empty((L1C, 128, O1), dtype=np.float16)
    for c in range(L1C):
        for q in range(4):
            m = (c + 16 * (q // 2)) % 32
            gbase = 32 * (q % 2)
            w1t[c, 32 * q : 32 * q + 32, :] = w1[:, gbase : gbase + 32, m].T.astype(
                np.float16
            )
    # [p, c, o] layout for contiguous per-partition DMA
    return (
        np.ascontiguousarray(w0t.transpose(1, 0, 2)),
        np.ascontiguousarray(w1t.transpose(1, 0, 2)),
        b0.astype(np.float32),
        b1.astype(np.float32),
    )


def _prep_inputs_core(x_core):
    """x_core: (BC, F, D) fp32 -> xa (N,128,K), xr0 (N,128,L0C,K),
    xb (N,128,L1C,K) fp16 operand tiles."""
    xcols = (
        x_core.reshape(NCHUNKS, BPC, F, D)
        .transpose(2, 0, 1, 3)
        .reshape(F, NCHUNKS, NCHUNK)
        .astype(np.float16)
    )
    xa = np.ascontiguousarray(xcols[ROWS_XA].transpose(1, 0, 2))
    xr0 = np.ascontiguousarray(xcols[ROWS_XR0].transpose(2, 0, 1, 3))
    xb = np.ascontiguousarray(xcols[ROWS_XB].transpose(2, 0, 1, 3))
    return xa, xr0, xb


# ---- kernel build ---------------------------------------------------------- #

_NC_CACHE = {}


def _build():
    nc = bacc.Bacc("TRN2", target_bir_lowering=False, debug=False)

    xa_d = nc.dram_tensor("xa", [NCHUNKS, 128, NCHUNK], DT, kind="ExternalInput")
    xr0_d = nc.dram_tensor(
        "xr0", [NCHUNKS, 128, L0C, NCHUNK], DT, kind="ExternalInput"
    )
    xb_d = nc.dram_tensor("xb", [NCHUNKS, 128, L1C, NCHUNK], DT, kind="ExternalInput")
    w0t_d = nc.dram_tensor("w0t", [128, L0C, O0], DT, kind="ExternalInput")
    w1t_d = nc.dram_tensor("w1t", [128, L1C, O1], DT, kind="ExternalInput")
    b0_d = nc.dram_tensor("b0", [O0, 1], FP32, kind="ExternalInput")
    b1_d = nc.dram_tensor("b1", [O1, 1], FP32, kind="ExternalInput")
    out_d = nc.dram_tensor("out", [BC, 192], FP32, kind="ExternalOutput")

    mult = mybir.AluOpType.mult

    with tile.TileContext(nc) as tc:
        with (
            tc.tile_pool(name="const", bufs=1) as cpool,
            tc.tile_pool(name="xin", bufs=2) as xpool,
            tc.tile_pool(name="xb", bufs=3) as xbpool,
            tc.tile_pool(name="z0", bufs=2) as z0pool,
            tc.tile_pool(name="y", bufs=2) as ypool,
            tc.tile_pool(name="f0", bufs=3) as f0pool,
            tc.tile_pool(name="f1", bufs=2) as f1pool,
            tc.tile_pool(name="psum", bufs=2, space="PSUM") as pspool,
        ):
            # resident weights, biases, accumulators, identity
            w0t = cpool.tile([128, L0C, O0], DT, tag="w0t")
            w1t = cpool.tile([128, L1C, O1], DT, tag="w1t")
            nc.sync.dma_start(w0t[:], w0t_d.ap())
            nc.sync.dma_start(w1t[:], w1t_d.ap())
            b0t = cpool.tile([O0, 1], FP32, tag="b0")
            b1t = cpool.tile([O1, 1], FP32, tag="b1")
            nc.sync.dma_start(b0t[:], b0_d.ap())
            nc.sync.dma_start(b1t[:], b1_d.ap())
            ident = cpool.tile([128, 128], FP32, tag="ident")
            make_identity(nc, ident[:])
            r0all = cpool.tile([128, BC], FP32, tag="r0all")  # rows 64:128 used
            r1all = cpool.tile([128, BC], FP32, tag="r1all")

            st = {}  # per-chunk live tiles

            def emit_dma(i):
                xa = xpool.tile([128, NCHUNK], DT, tag="xa", name=f"xa_{i}")
                nc.sync.dma_start(xa[:], xa_d.ap()[i])
                xr0 = xpool.tile([128, L0C, NCHUNK], DT, tag="xr0", name=f"xr0_{i}")
                nc.sync.dma_start(xr0[:], xr0_d.ap()[i])
                xb = xbpool.tile([128, L1C, NCHUNK], DT, tag="xb", name=f"xb_{i}")
                for c in range(L1C):
                    eng = nc.scalar if c < XB_SPLIT else nc.sync
                    eng.dma_start(xb[:, c], xb_d.ap()[i][:, c])
                st[i] = {"xa": xa, "xr0": xr0, "xb": xb}

            def emit_l0a(i):
                s = st[i]
                xa, xr0 = s["xa"], s["xr0"]
                z0 = z0pool.tile([128, L0C, NCHUNK], DT, tag="z0", name=f"z0_{i}")
                xa_b = xa[:].unsqueeze(1).broadcast_to((128, L0C, NCHUNK))
                nc.vector.tensor_tensor(z0[:], xr0[:], xa_b, op=mult)
                ps0 = pspool.tile([128, NCHUNK], FP32, tag="ps0", name=f"ps0_{i}")
                for c in range(L0C):
                    for sl in range(NCHUNK // MMF):
                        nc.tensor.matmul(
                            ps0[:, sl * MMF : (sl + 1) * MMF],
                            w0t[:, c],
                            z0[:, c, sl * MMF : (sl + 1) * MMF],
                            start=(c == 0),
                            stop=(c == L0C - 1),
                        )
                y2d = ypool.tile([128, NCHUNK], DT, tag="y2d", name=f"y2d_{i}")
                nc.scalar.activation(
                    y2d[:H1],
                    ps0[:H1],
                    mybir.ActivationFunctionType.Relu,
                    bias=b0t[:H1],
                )
                f0s = f0pool.tile([128, BPC, D], DT, tag="f0s", name=f"f0s_{i}")
                nc.scalar.activation(
                    f0s[H1:128].rearrange("p a b -> p (a b)"),
                    ps0[H1:128],
                    mybir.ActivationFunctionType.Relu,
                    bias=b0t[H1:128],
                )
                s["z0"] = z0
                s["ps0"] = ps0
                s["y2d"] = y2d
                s["f0s"] = f0s

            def emit_l0b(i):
                # duplicate y into the upper half for the z1 multiplies, on
                # the Scalar engine (partition-shifted copy; DVE stays free)
                y2d = st[i]["y2d"]
                nc.scalar.activation(
                    y2d[H1:128], y2d[:H1], mybir.ActivationFunctionType.Copy
                )

            def emit_l1(i):
                s = st[i]
                xb, y2d = s["xb"], s["y2d"]
                for a, b in DVE_GROUPS:
                    y_b = y2d[:].unsqueeze(1).broadcast_to((128, b - a, NCHUNK))
                    nc.vector.tensor_tensor(xb[:, a:b], xb[:, a:b], y_b, op=mult)
                for c in POOL_CHUNKS:
                    nc.gpsimd.tensor_tensor(xb[:, c], xb[:, c], y2d[:], op=mult)
                ps1 = pspool.tile([128, NCHUNK], FP32, tag="ps1", name=f"ps1_{i}")
                for c in range(L1C):
                    for sl in range(NCHUNK // MMF):
                        nc.tensor.matmul(
                            ps1[:, sl * MMF : (sl + 1) * MMF],
                            w1t[:, c],
                            xb[:, c, sl * MMF : (sl + 1) * MMF],
                            start=(c == 0),
                            stop=(c == L1C - 1),
                        )
                f1s = f1pool.tile([128, BPC, D], DT, tag="f1s", name=f"f1s_{i}")
                nc.scalar.activation(
                    f1s[:].rearrange("p a b -> p (a b)"),
                    ps1[:],
                    mybir.ActivationFunctionType.Relu,
                    bias=b1t[:],
                )
                s["f1s"] = f1s

            def emit_red0(i):
                cols = slice(i * BPC, (i + 1) * BPC)
                nc.vector.tensor_reduce(
                    r0all[H1:128, cols],
                    st[i]["f0s"][H1:128],
                    axis=mybir.AxisListType.X,
                    op=mybir.AluOpType.add,
                )

            def emit_red1(i):
                cols = slice(i * BPC, (i + 1) * BPC)
                nc.vector.tensor_reduce(
                    r1all[:, cols],
                    st[i]["f1s"][:],
                    axis=mybir.AxisListType.X,
                    op=mybir.AluOpType.add,
                )
                del st[i]

            # ---- final transpose (channel, batch) -> (batch, channel) ----
            # seg 0 (batch 0:128 = chunks 0:8) is emitted as soon as those
            # chunks are reduced, overlapping the tail of the chunk loop.
            outbuf = cpool.tile([128, 2, 192], FP32, tag="outbuf")

            def emit_out_seg(seg):
                cs = slice(seg * 128, (seg + 1) * 128)
                pt1 = pspool.tile([128, 128], FP32, tag="ps1", name=f"pt1_{seg}")
                nc.tensor.matmul(
                    pt1[:], r1all[:, cs], ident[:], is_transpose=True
                )
                nc.scalar.activation(
                    outbuf[:, seg, H1:192],
                    pt1[:],
                    mybir.ActivationFunctionType.Copy,
                )
                pt0 = pspool.tile([128, 64], FP32, tag="ps0", name=f"pt0_{seg}")
                nc.tensor.matmul(
                    pt0[:],
                    r0all[H1:128, cs],
                    ident[H1:128, H1:128],
                    is_transpose=True,
                )
                nc.scalar.activation(
                    outbuf[:, seg, 0:H1],
                    pt0[:],
                    mybir.ActivationFunctionType.Copy,
                )
                nc.sync.dma_start(
                    out_d.ap().rearrange("(s b) c -> b s c", s=2)[:, seg],
                    outbuf[:, seg],
                )

            for p in range(NCHUNKS + 5):
                if p < NCHUNKS:
                    emit_dma(p)
                if 1 <= p and p - 1 < NCHUNKS:
                    emit_l0a(p - 1)
                    emit_l0b(p - 1)
                if 2 <= p and p - 2 < NCHUNKS:
                    emit_l1(p - 2)
                    emit_red0(p - 2)
                if 3 <= p and p - 3 < NCHUNKS:
                    emit_red1(p - 3)
                if p - 3 == NCHUNKS // 2 - 1:
                    emit_out_seg(0)
            emit_out_seg(1)

    nc.compile()
    return nc


def _get_nc():
    if "nc" not in _NC_CACHE:
        _NC_CACHE["nc"] = _build()
    return _NC_CACHE["nc"]


def _make_in_maps(inputs, W0, b0, W1, b1):
    w0t, w1t, b0f, b1f = _prep_weights(W0, b0, W1, b1)
    in_maps = []
    for core in range(NCORES):
        xa, xr0, xb = _prep_inputs_core(inputs[core * BC : (core + 1) * BC])
        in_maps.append(
            {
                "xa": xa,
                "xr0": xr0,
                "xb": xb,
                "w0t": w0t,
                "w1t": w1t,
                "b0": b0f[:, None],
                "b1": b1f[:, None],
            }
        )
    return in_maps


def kernel(inputs, W0, b0, W1, b1):
    inputs = np.asarray(inputs, dtype=np.float32)
    nc = _get_nc()
    in_maps = _make_in_maps(
        inputs,
        np.asarray(W0, np.float32),
        np.asarray(b0, np.float32),
        np.asarray(W1, np.float32),
        np.asarray(b1, np.float32),
    )
    res = run_bass_kernel_spmd(nc, in_maps, core_ids=list(range(NCORES)))
    out = np.concatenate([res.results[c]["out"] for c in range(NCORES)], axis=0)
    return out.astype(np.float32)


def _install_ntff_hook():
    """The container's antenv lacks axon_hooks; synthesize it around the
    injected libaxon_pjrt.so so run_bass_kernel_spmd(trace=True) works."""
    import types

    if "antenv.axon_hooks" in sys.modules:
        return
    sys.path.insert(0, "/root/.axon_site")
    from trn_agent_boot.trn_boot import _ntff_profile_via_ctypes

    hook = _ntff_profile_via_ctypes("/opt/axon/libaxon_pjrt.so")
    m = types.ModuleType("antenv.axon_hooks")
    m.get_axon_ntff_profile_hook = lambda: hook
    m.set_axon_ntff_profile_hook = lambda h: None
    sys.modules["antenv.axon_hooks"] = m


def profile_once(inputs_dict, tmpdir=None):
    """Run once with NTFF tracing; return exec_time_ns (core 0)."""
    _install_ntff_hook()
    nc = _get_nc()
    in_maps = _make_in_maps(
        np.asarray(inputs_dict["inputs"], np.float32),
        np.asarray(inputs_dict["W0"], np.float32),
        np.asarray(inputs_dict["b0"], np.float32),
        np.asarray(inputs_dict["W1"], np.float32),
        np.asarray(inputs_dict["b1"], np.float32),
    )
    res = run_bass_kernel_spmd(
        nc, in_maps, core_ids=list(range(NCORES)), trace=True, tmpdir=tmpdir
    )
    return res.exec_time_ns


if __name__ == "__main__":
    rng = np.random.default_rng(0)
    inputs = rng.standard_normal((B, F, D), dtype=np.float32)
    W0 = (rng.standard_normal((O0, F * F), dtype=np.float32) * 0.03).astype(np.float32)
    W1 = (rng.standard_normal((O1, H1 * F), dtype=np.float32) * 0.03).astype(np.float32)
    b0 = np.zeros(O0, np.float32)
    b1 = np.zeros(O1, np.float32)
    out = kernel(inputs=inputs, W0=W0, b0=b0, W1=W1, b1=b1)
    print("kernel out", out.shape, out.dtype, out[:2, :4])
